# revision 18
# baseline (speedup 1.0000x reference)
"""Trainium2 Bass kernel for spatial-reduction attention (nn_Attention_11269994184820).

Strategy: head-parallel over 8 cores (8 heads). Each core computes one head's
attention for all 4 batches in a transposed layout, then chunked AllToAlls
(overlapped with compute) redistribute head-outputs to token-slices, and each
core applies the output projection for its 2048 token rows.

Key points:
  - exp(qk + rel) = exp(qk) * exp(rel): exp(rel) precomputed on HOST in fp16;
    on-chip: ACT exp straight from PSUM (fp32->fp16), then fp16 multiply in
    DVE 2x mode (a slice of the multiplies goes to GpSimd to balance load).
  - all PE-path data fp16: 1 cycle/col matmuls, halved DMA for x and rel.
  - rel loaded ONCE, host-tiled into contiguous fp16 blocks per (qc, kc).
  - score matmuls row-packed via tile_position (concurrent PE sub-tiles);
    AV matmuls col-packed 2 batches per PSUM bank (rows 0:33 / 64:97).
  - AllToAll split into 4 chunks {qc k, qc k+4}, launched as soon as ready;
    normalize+projection per chunk overlaps the remaining compute.
  - batch prep (conv/q/k/v) emission interleaved with first-chunk attention
    so the scalar engine starts exp-ing early.

Layouts (per core = head h):
  - xT[b]      [256, 4096] fp16  x transposed (host prep), 2 c-chunks of 128
  - erel       [8 qc][8 kc][128, 512] fp16 = exp(rel[h]).T host-tiled
  - qrep       [128, 4096] fp16  qT replicated 4x along partitions
  - kstrip     [128, 128]x2 fp16 kT chunks at partition strips
  - scores     PSUM [128 k, 1024 = 2kc x 512 q] fp32, row-packed QK matmuls
  - et         ACT Exp PSUM->SBUF fp16, then *= erel (DVE/Pool 2x)
  - AV         [v|1] over 8 kc -> av PSUM rows 0:33 (b even) / 64:97 (b odd)
  - AllToAll   4 chunks of [8, 33, 512] fp32; normalize + proj per chunk
"""

import sys

if "/opt/trn_rl_repo" not in sys.path:
    sys.path.insert(0, "/opt/trn_rl_repo")

from contextlib import ExitStack

import numpy as np

import concourse.bacc as bacc
import concourse.bass as bass
import concourse.mybir as mybir
import concourse.tile as tile
from concourse.bass_utils import run_bass_kernel_spmd

F32 = mybir.dt.float32
FP16 = mybir.dt.float16
N_CORES = 8
B, N, C = 4, 4096, 256
HEADS, DH, SR, NK = 8, 32, 2, 1024
BN_EPS = 1e-5

_CACHE = {}


def _build_nc():
    nc = bacc.Bacc("TRN2", target_bir_lowering=False, debug=False, num_devices=N_CORES)

    def din(name, shape, dt=FP16):
        return nc.dram_tensor(name, list(shape), dt, kind="ExternalInput").ap()

    xt_d = din("xt", [B, 2, 128, N])
    erel_d = din("erel", [8, 8, 128, 512])
    qw_d = din("qw", [2, 128, 128])
    kw_d = din("kw", [2, 128, 32])
    vw_d = din("vw", [2, 128, 32])
    cw_d = din("cw", [4, 2, 128, 128])
    bna_d = din("bna", [2, 128, 1], F32)
    bnb_d = din("bnb", [2, 128, 1], F32)
    pwt_d = din("pwt", [2, 128, 256])
    pb_d = din("pbrep", [128, 256], F32)
    out_d = nc.dram_tensor("out", [2048, 256], F32, kind="ExternalOutput").ap()

    with tile.TileContext(nc) as tc, ExitStack() as ctx:
        pool = ctx.enter_context(tc.tile_pool(name="main", bufs=1))
        p_dram = ctx.enter_context(tc.tile_pool(name="dram", bufs=1, space="DRAM"))
        ps_sc = ctx.enter_context(tc.tile_pool(name="ps_sc", bufs=2, space="PSUM"))
        ps_av = ctx.enter_context(tc.tile_pool(name="ps_av", bufs=1, space="PSUM"))
        ps_mi = ctx.enter_context(tc.tile_pool(name="ps_mi", bufs=2, space="PSUM"))

        # chunked collective buffers (recv in Shared space for HBM-HBM cc)
        outu_d = [p_dram.tile([8, 33, 512], F32, tag=f"outu{k}", name=f"outu{k}")
                  for k in range(4)]
        recv_d = [p_dram.tile([8, 33, 512], F32, tag=f"recv{k}", name=f"recv{k}")
                  for k in range(4)]
        recip_d = p_dram.tile([4, 8, 512], F32, tag="recipd", name="recipd")

        # ---- constants into SBUF ----
        def const_tile(src, shape, tag, dt=FP16):
            t = pool.tile(shape, dt, tag=tag)
            nc.sync.dma_start(t[:], src)
            return t

        # HAM warmup: ~24 back-to-back matmuls flip the PE clock gate to 8/8
        # (2.4 GHz) during the initial x-DMA wait; steady-state gaps are too
        # short to re-throttle afterwards.
        wz = pool.tile([128, 512], FP16, tag="wz")
        nc.vector.memset(wz[:], 0.0)
        for w in range(24):
            psw = ps_mi.tile([128, 512], F32, tag="mi", name=f"warm{w}")
            nc.tensor.matmul(psw[:], wz[:, 0:128], wz[:], start=True, stop=True)

        qw_sb = [const_tile(qw_d[cc], [128, 128], f"qw{cc}") for cc in range(2)]
        kw_sb = [const_tile(kw_d[cc], [128, 32], f"kw{cc}") for cc in range(2)]
        vw_sb = [const_tile(vw_d[cc], [128, 32], f"vw{cc}") for cc in range(2)]
        cw_sb = [[const_tile(cw_d[t, cc], [128, 128], f"cw{t}{cc}")
                  for cc in range(2)] for t in range(4)]
        bna_sb = [const_tile(bna_d[cc], [128, 1], f"bna{cc}", F32) for cc in range(2)]
        bnb_sb = [const_tile(bnb_d[cc], [128, 1], f"bnb{cc}", F32) for cc in range(2)]
        pwt_sb = [const_tile(pwt_d[cc], [128, 256], f"pwt{cc}") for cc in range(2)]
        pb_sb = const_tile(pb_d[:], [128, 256], "pbrep", F32)

        bctx = [None] * B

        def prep_batch(b):
            xt_sb = []
            for cc in range(2):
                t = pool.tile([128, N], FP16, tag=f"xt{cc}", bufs=2,
                              name=f"xt{b}{cc}")
                nc.scalar.dma_start(t[:], xt_d[b, cc])
                xt_sb.append(t)

            # depthwise 2x2/2 conv as 4 diag matmuls + BN fold on evacuation
            xkbn = []
            for cc in range(2):
                xk = pool.tile([128, NK], FP16, tag=f"xkbn{cc}", bufs=2,
                               name=f"xkbn{b}{cc}")
                view = xt_sb[cc][:].rearrange(
                    "p (i a j b) -> p i a j b", i=32, a=2, j=32, b=2
                )
                for half in range(2):
                    psc = ps_mi.tile([128, 512], F32, tag="mi", name=f"cv{b}{cc}{half}")
                    for t in range(4):
                        di, dj = t // 2, t % 2
                        rhs = view[:, half * 16:(half + 1) * 16, di, :, dj]
                        nc.tensor.matmul(psc[:], cw_sb[t][cc][:], rhs,
                                         start=(t == 0), stop=(t == 3))
                    nc.vector.tensor_scalar(
                        xk[:, half * 512:(half + 1) * 512], psc[:],
                        bna_sb[cc][:], bnb_sb[cc][:],
                        op0=mybir.AluOpType.mult, op1=mybir.AluOpType.add)
                xkbn.append(xk)

            # q projection (replicated 4x along partitions), fp16 output
            qrep = pool.tile([128, N], FP16, tag=f"qrep{b}", name=f"qrep{b}")
            for ncc in range(8):
                psq = ps_mi.tile([128, 512], F32, tag="mi", name=f"q{b}{ncc}")
                for cc in range(2):
                    nc.tensor.matmul(psq[:], qw_sb[cc][:],
                                     xt_sb[cc][:, ncc * 512:(ncc + 1) * 512],
                                     start=(cc == 0), stop=(cc == 1))
                nc.vector.tensor_copy(qrep[:, ncc * 512:(ncc + 1) * 512], psq[:])

            # k projection into partition strips (fp16)
            kstrip = []
            for grp in range(2):
                psk = ps_mi.tile([128, 128], F32, tag="mi", name=f"k{b}{grp}")
                for s in range(4):
                    kc = grp * 4 + s
                    for cc in range(2):
                        nc.tensor.matmul(
                            psk[32 * s:32 * (s + 1), :],
                            kw_sb[cc][:],
                            xkbn[cc][:, kc * 128:(kc + 1) * 128],
                            start=(cc == 0), stop=(cc == 1),
                            tile_position=(0, 32 * s))
                kt = pool.tile([128, 128], FP16, tag=f"ks{b}{grp}", name=f"ks{b}{grp}")
                nc.vector.tensor_copy(kt[:], psk[:])
                kstrip.append(kt)

            # v projection: all 8 kc into one [128, 256] PSUM tile, then one
            # strided copy into [128, 8*33] (+ ones columns) for AV stationary
            psv = ps_mi.tile([128, 256], F32, tag="mi", name=f"v{b}")
            for kc in range(8):
                for cc in range(2):
                    nc.tensor.matmul(
                        psv[:, kc * 32:(kc + 1) * 32],
                        xkbn[cc][:, kc * 128:(kc + 1) * 128],
                        vw_sb[cc][:],
                        start=(cc == 0), stop=(cc == 1))
            vsb = pool.tile([128, 8 * 33], FP16, tag=f"v{b}", name=f"vt{b}")
            vview = vsb[:].rearrange("p (k d) -> p k d", k=8, d=33)
            nc.vector.tensor_copy(
                vview[:, :, 0:32],
                psv[:].rearrange("p (k d) -> p k d", k=8, d=32))
            nc.vector.memset(vview[:, :, 32:33], 1.0)
            bctx[b] = (qrep, kstrip, vsb)

        # erel tiles: one [128, 4096] fp16 tile per qc (8 contiguous 128KB DMAs)
        def load_erel(qc):
            er = pool.tile([128, 8 * 512], FP16, tag=f"erel{qc % 2}", bufs=2,
                           name=f"erel{qc}")
            for kc in range(8):
                nc.sync.dma_start(er[:, kc * 512:(kc + 1) * 512], erel_d[qc, kc])
            return er

        def attn(qc, b, er, av_tile):
            qrep, kstrip, vsb = bctx[b]
            off = 64 * (b % 2)
            qsl = slice(qc * 512, (qc + 1) * 512)
            et = pool.tile([128, 4096], FP16, tag="et", bufs=2,
                           name=f"et{qc}{b}")
            for g in range(4):
                pssc = ps_sc.tile([128, 1024], F32, tag="sc", name=f"sc{qc}{b}{g}")
                for u in range(2):
                    kc = 2 * g + u
                    s = kc % 4
                    nc.tensor.matmul(
                        pssc[:, u * 512:(u + 1) * 512],
                        kstrip[kc // 4][32 * s:32 * (s + 1), :],
                        qrep[32 * s:32 * (s + 1), qsl],
                        start=True, stop=True,
                        tile_position=(32 * s, 0))
                nc.scalar.activation(et[:, g * 1024:(g + 1) * 1024], pssc[:],
                                     mybir.ActivationFunctionType.Exp)
            # one big fp16 2x-mode multiply for the whole (qc, b) tile; a few
            # late-chunk multiplies go to the (otherwise idle) Pool engine,
            # placed where they cannot queue behind a collective
            meng = nc.gpsimd if (qc >= 4 and b % 2 == 1) else nc.vector
            meng.tensor_tensor(et[:], et[:], er[:],
                               op=mybir.AluOpType.mult)
            for kc in range(8):
                nc.tensor.matmul(
                    av_tile[off:off + 33, :],
                    vsb[:, kc * 33:kc * 33 + 33],
                    et[:, kc * 512:(kc + 1) * 512],
                    start=(kc == 0), stop=(kc == 7),
                    tile_position=(0, off), skip_group_check=True)

        def emit_out(qc, bpair, av_tile):
            ou = pool.tile([97, 512], F32, tag="outu", bufs=3, name=f"ou{qc}{bpair}")
            for b in (2 * bpair, 2 * bpair + 1):
                off = 64 * (b % 2)
                nc.vector.tensor_copy(ou[off:off + 33, :], av_tile[off:off + 33, :])
                dest = b * 2 + qc // 4
                nc.sync.dma_start(outu_d[qc % 4][dest], ou[off:off + 33, :])

        # qc iteration order: chunk k = {qc k, qc k+4} -> covers all 8 dests
        CHUNK_QCS = [(0, 4), (1, 5), (2, 6), (3, 7)]

        def attn_qc(qc, bs, er):
            for b in bs:
                if b % 2 == 0:
                    av = ps_av.tile([128, 512], F32, tag=f"av{b // 2}",
                                    name=f"av{qc}p{b // 2}")
                    attn_qc.av[b // 2] = av
                attn(qc, b, er, attn_qc.av[b // 2])
            for bpair in sorted({b // 2 for b in bs}):
                emit_out(qc, bpair, attn_qc.av[bpair])

        attn_qc.av = [None, None]

        def launch_chunk(k):
            nc.gpsimd.collective_compute(
                "AllToAll", mybir.AluOpType.bypass,
                replica_groups=[list(range(N_CORES))],
                ins=[outu_d[k].opt()], outs=[recv_d[k].opt()])

        def proj_chunk(k):
            rv = recv_d[k]
            # denominators packed [128, 32]: head s -> partitions 16s..16s+16
            den = pool.tile([128, 32], F32, tag="den", bufs=2, name=f"den{k}")
            for s in range(8):
                nc.sync.dma_start(
                    den[16 * s:16 * (s + 1), :],
                    rv[s, 32, :].rearrange("(a c) -> a c", a=16, c=32))
            recip = pool.tile([128, 32], F32, tag="recip", bufs=2, name=f"rcp{k}")
            nc.vector.reciprocal(recip[:], den[:])
            nc.sync.dma_start(recip_d[k], recip[:])

            lhs = [pool.tile([128, 512], F32, tag=f"lhs{i}", bufs=2,
                             name=f"lhs{k}{i}") for i in range(2)]
            for s in range(8):
                nc.sync.dma_start(lhs[s // 4][(s % 4) * 32:(s % 4 + 1) * 32, :],
                                  rv[s, 0:32, :])
            bcr = [pool.tile([128, 512], F32, tag=f"bcr{i}", bufs=2,
                             name=f"bcr{k}{i}") for i in range(2)]
            for s in range(8):
                nc.gpsimd.dma_start(
                    bcr[s // 4][(s % 4) * 32:(s % 4 + 1) * 32, :],
                    recip_d[k, s, :].partition_broadcast(32))
            lhsh = [pool.tile([128, 512], FP16, tag=f"lhsh{i}", bufs=2,
                              name=f"lhsh{k}{i}") for i in range(2)]
            for i in range(2):
                nc.gpsimd.tensor_tensor(lhsh[i][:], lhs[i][:], bcr[i][:],
                                        op=mybir.AluOpType.mult)
            for r in range(4):
                psp = ps_mi.tile([128, 256], F32, tag="mi", name=f"pj{k}{r}")
                for i in range(2):
                    nc.tensor.matmul(psp[:], lhsh[i][:, r * 128:(r + 1) * 128],
                                     pwt_sb[i][:],
                                     start=(i == 0), stop=(i == 1))
                ot = pool.tile([128, 256], F32, tag="ot", bufs=2,
                               name=f"ot{k}{r}")
                nc.vector.tensor_add(ot[:], psp[:], pb_sb[:])
                nc.sync.dma_start(out_d[k * 512 + r * 128:k * 512 + (r + 1) * 128, :],
                                  ot[:])

        # ---- prologue: interleave batch prep with first-chunk attention ----
        prep_batch(0)
        prep_batch(1)
        er0 = load_erel(0)
        attn_qc(0, [0, 1], er0)
        er4 = load_erel(4)
        attn_qc(4, [0, 1], er4)
        prep_batch(2)
        prep_batch(3)
        attn_qc(0, [2, 3], er0)
        attn_qc(4, [2, 3], er4)
        launch_chunk(0)

        for k in range(1, 4):
            for qc in CHUNK_QCS[k]:
                er = load_erel(qc)
                attn_qc(qc, [0, 1, 2, 3], er)
            if k >= 2:
                proj_chunk(k - 2)
            if k == 3:
                proj_chunk(2)
            launch_chunk(k)
        proj_chunk(3)

    nc.compile()
    return nc


def _host_prep(x, relative_pos, q_w, k_w, v_w, proj_w, proj_b, sr_w, sr_b,
               bn_gamma, bn_beta, bn_mean, bn_var):
    f = np.float32
    h16 = np.float16
    x = np.asarray(x, f)
    relative_pos = np.asarray(relative_pos, f)
    scale = np.float32(DH ** -0.5)

    xt = np.ascontiguousarray(x.transpose(0, 2, 1)).reshape(B, 2, 128, N)
    xt = xt.astype(h16)
    a = (np.asarray(bn_gamma, f) / np.sqrt(np.asarray(bn_var, f) + BN_EPS)).astype(f)
    b_eff = ((np.asarray(sr_b, f) - np.asarray(bn_mean, f)) * a
             + np.asarray(bn_beta, f)).astype(f)
    sr_w = np.asarray(sr_w, f)
    cw = np.zeros((4, 2, 128, 128), h16)
    for t in range(4):
        tap = sr_w[:, 0, t // 2, t % 2]
        for cc in range(2):
            cw[t, cc] = np.diag(tap[cc * 128:(cc + 1) * 128]).astype(h16)
    bna = a.reshape(2, 128, 1).astype(f)
    bnb = b_eff.reshape(2, 128, 1).astype(f)
    pwt = np.ascontiguousarray(np.asarray(proj_w, f).T).reshape(2, 128, 256)
    pwt = pwt.astype(h16)
    pbrep = np.tile(np.asarray(proj_b, f).reshape(1, 256), (128, 1))

    in_maps = []
    for h in range(N_CORES):
        qwT_rep = np.tile(
            np.ascontiguousarray((np.asarray(q_w, f)[h * 32:(h + 1) * 32, :]
                                  * scale).T), (1, 4)).reshape(2, 128, 128)
        kwT = np.ascontiguousarray(
            np.asarray(k_w, f)[h * 32:(h + 1) * 32, :].T).reshape(2, 128, 32)
        vwT = np.ascontiguousarray(
            np.asarray(v_w, f)[h * 32:(h + 1) * 32, :].T).reshape(2, 128, 32)
        # erel[qc, kc, p, c] = exp(rel[h][qc*512 + c, kc*128 + p]) in fp16
        r = relative_pos[h].reshape(8, 512, 8, 128)
        erel = np.exp(r).transpose(0, 2, 3, 1).astype(h16)
        in_maps.append({
            "xt": xt, "erel": np.ascontiguousarray(erel),
            "qw": np.ascontiguousarray(qwT_rep).astype(h16),
            "kw": kwT.astype(h16), "vw": vwT.astype(h16), "cw": cw,
            "bna": bna, "bnb": bnb,
            "pwt": np.ascontiguousarray(pwt),
            "pbrep": np.ascontiguousarray(pbrep),
        })
    return in_maps


def run_once(inputs, trace=False, trace_kwargs=None):
    if trace:
        try:
            import antenv.axon_hooks  # noqa: F401
        except ImportError:
            trace = False
    if "nc" not in _CACHE:
        _CACHE["nc"] = _build_nc()
    nc = _CACHE["nc"]
    in_maps = _host_prep(
        inputs["x"], inputs["relative_pos"], inputs["q_w"], inputs["k_w"],
        inputs["v_w"], inputs["proj_w"], inputs["proj_b"], inputs["sr_w"],
        inputs["sr_b"], inputs["bn_gamma"], inputs["bn_beta"],
        inputs["bn_mean"], inputs["bn_var"])
    res = run_bass_kernel_spmd(nc, in_maps, core_ids=list(range(N_CORES)),
                               trace=trace, **(trace_kwargs or {}))
    out = np.zeros((B, N, C), np.float32)
    for i in range(N_CORES):
        bb, nh = i // 2, i % 2
        out[bb, nh * 2048:(nh + 1) * 2048, :] = res.results[i]["out"]
    return out, res


def kernel(**inputs) -> np.ndarray:
    out, _ = run_once(inputs, trace=False)
    return out


# revision 19
# speedup vs baseline: 1.2616x; 1.2616x over previous
"""Trainium2 Bass kernel for spatial-reduction attention (nn_Attention_11269994184820).

Strategy: head-parallel over 8 cores (8 heads). Each core computes one head's
attention for all 4 batches in a transposed layout, then chunked AllToAlls
(overlapped with compute) redistribute head-outputs to token-slices, and each
core applies the output projection for its 2048 token rows.

Key points:
  - exp(qk + rel) = exp(qk) * exp(rel): exp(rel) precomputed on HOST in fp16;
    on-chip: ACT exp straight from PSUM (fp32->fp16), then fp16 multiply in
    DVE 2x mode (a slice of the multiplies goes to GpSimd to balance load).
  - all PE-path data fp16: 1 cycle/col matmuls, halved DMA for x and rel.
  - rel loaded ONCE, host-tiled into contiguous fp16 blocks per (qc, kc).
  - score matmuls row-packed via tile_position (concurrent PE sub-tiles);
    AV matmuls col-packed 2 batches per PSUM bank (rows 0:33 / 64:97).
  - AllToAll split into 4 chunks {qc k, qc k+4}, launched as soon as ready;
    normalize+projection per chunk overlaps the remaining compute.
  - batch prep (conv/q/k/v) emission interleaved with first-chunk attention
    so the scalar engine starts exp-ing early.

Layouts (per core = head h):
  - xT[b]      [256, 4096] fp16  x transposed (host prep), 2 c-chunks of 128
  - erel       [8 qc][8 kc][128, 512] fp16 = exp(rel[h]).T host-tiled
  - qrep       [128, 4096] fp16  qT replicated 4x along partitions
  - kstrip     [128, 128]x2 fp16 kT chunks at partition strips
  - scores     PSUM [128 k, 1024 = 2kc x 512 q] fp32, row-packed QK matmuls
  - et         ACT Exp PSUM->SBUF fp16, then *= erel (DVE/Pool 2x)
  - AV         [v|1] over 8 kc -> av PSUM rows 0:33 (b even) / 64:97 (b odd)
  - AllToAll   4 chunks of [8, 33, 512] fp32; normalize + proj per chunk
"""

import sys

if "/opt/trn_rl_repo" not in sys.path:
    sys.path.insert(0, "/opt/trn_rl_repo")

from contextlib import ExitStack

import numpy as np

import concourse.bacc as bacc
import concourse.bass as bass
import concourse.mybir as mybir
import concourse.tile as tile
from concourse.bass_utils import run_bass_kernel_spmd

F32 = mybir.dt.float32
FP16 = mybir.dt.float16
N_CORES = 8
B, N, C = 4, 4096, 256
HEADS, DH, SR, NK = 8, 32, 2, 1024
BN_EPS = 1e-5

_CACHE = {}


def _build_nc():
    nc = bacc.Bacc("TRN2", target_bir_lowering=False, debug=False, num_devices=N_CORES)

    def din(name, shape, dt=FP16):
        return nc.dram_tensor(name, list(shape), dt, kind="ExternalInput").ap()

    xt_d = din("xt", [B, 2, 128, N])
    erel_d = din("erel", [8, 8, 128, 512])
    qw_d = din("qw", [2, 128, 128])
    kw_d = din("kw", [2, 128, 32])
    vw_d = din("vw", [2, 128, 32])
    cw_d = din("cw", [4, 2, 128, 128])
    bna_d = din("bna", [2, 128, 1], F32)
    bnb_d = din("bnb", [2, 128, 1], F32)
    pwt_d = din("pwt", [2, 128, 256])
    pb_d = din("pbrep", [128, 256], F32)
    out_d = nc.dram_tensor("out", [2048, 256], F32, kind="ExternalOutput").ap()

    with tile.TileContext(nc) as tc, ExitStack() as ctx:
        pool = ctx.enter_context(tc.tile_pool(name="main", bufs=1))
        p_dram = ctx.enter_context(tc.tile_pool(name="dram", bufs=1, space="DRAM"))
        ps_sc = ctx.enter_context(tc.tile_pool(name="ps_sc", bufs=2, space="PSUM"))
        ps_av = ctx.enter_context(tc.tile_pool(name="ps_av", bufs=1, space="PSUM"))
        ps_mi = ctx.enter_context(tc.tile_pool(name="ps_mi", bufs=2, space="PSUM"))

        # chunked collective buffers (recv in Shared space for HBM-HBM cc)
        outu_d = [p_dram.tile([8, 33, 512], F32, tag=f"outu{k}", name=f"outu{k}")
                  for k in range(4)]
        recv_d = [p_dram.tile([8, 33, 512], F32, tag=f"recv{k}", name=f"recv{k}")
                  for k in range(4)]
        recip_d = p_dram.tile([4, 8, 512], F32, tag="recipd", name="recipd")

        # ---- constants into SBUF ----
        def const_tile(src, shape, tag, dt=FP16):
            t = pool.tile(shape, dt, tag=tag)
            nc.sync.dma_start(t[:], src)
            return t

        # HAM warmup: ~24 back-to-back matmuls flip the PE clock gate to 8/8
        # (2.4 GHz) during the initial x-DMA wait; steady-state gaps are too
        # short to re-throttle afterwards.
        wz = pool.tile([128, 512], FP16, tag="wz")
        nc.vector.memset(wz[:], 0.0)
        for w in range(24):
            psw = ps_mi.tile([128, 512], F32, tag="mi", name=f"warm{w}")
            nc.tensor.matmul(psw[:], wz[:, 0:128], wz[:], start=True, stop=True)

        qw_sb = [const_tile(qw_d[cc], [128, 128], f"qw{cc}") for cc in range(2)]
        kw_sb = [const_tile(kw_d[cc], [128, 32], f"kw{cc}") for cc in range(2)]
        vw_sb = [const_tile(vw_d[cc], [128, 32], f"vw{cc}") for cc in range(2)]
        cw_sb = [[const_tile(cw_d[t, cc], [128, 128], f"cw{t}{cc}")
                  for cc in range(2)] for t in range(4)]
        bna_sb = [const_tile(bna_d[cc], [128, 1], f"bna{cc}", F32) for cc in range(2)]
        bnb_sb = [const_tile(bnb_d[cc], [128, 1], f"bnb{cc}", F32) for cc in range(2)]
        pwt_sb = [const_tile(pwt_d[cc], [128, 256], f"pwt{cc}") for cc in range(2)]
        pb_sb = const_tile(pb_d[:], [128, 256], "pbrep", F32)

        bctx = [None] * B

        def prep_batch(b):
            xt_sb = []
            for cc in range(2):
                t = pool.tile([128, N], FP16, tag=f"xt{cc}", bufs=2,
                              name=f"xt{b}{cc}")
                nc.scalar.dma_start(t[:], xt_d[b, cc])
                xt_sb.append(t)

            # depthwise 2x2/2 conv as 4 diag matmuls + BN fold on evacuation
            xkbn = []
            for cc in range(2):
                xk = pool.tile([128, NK], FP16, tag=f"xkbn{cc}", bufs=2,
                               name=f"xkbn{b}{cc}")
                view = xt_sb[cc][:].rearrange(
                    "p (i a j b) -> p i a j b", i=32, a=2, j=32, b=2
                )
                for half in range(2):
                    psc = ps_mi.tile([128, 512], F32, tag="mi", name=f"cv{b}{cc}{half}")
                    for t in range(4):
                        di, dj = t // 2, t % 2
                        rhs = view[:, half * 16:(half + 1) * 16, di, :, dj]
                        nc.tensor.matmul(psc[:], cw_sb[t][cc][:], rhs,
                                         start=(t == 0), stop=(t == 3))
                    nc.vector.tensor_scalar(
                        xk[:, half * 512:(half + 1) * 512], psc[:],
                        bna_sb[cc][:], bnb_sb[cc][:],
                        op0=mybir.AluOpType.mult, op1=mybir.AluOpType.add)
                xkbn.append(xk)

            # q projection (replicated 4x along partitions), fp16 output
            qrep = pool.tile([128, N], FP16, tag=f"qrep{b}", name=f"qrep{b}")
            for ncc in range(8):
                psq = ps_mi.tile([128, 512], F32, tag="mi", name=f"q{b}{ncc}")
                for cc in range(2):
                    nc.tensor.matmul(psq[:], qw_sb[cc][:],
                                     xt_sb[cc][:, ncc * 512:(ncc + 1) * 512],
                                     start=(cc == 0), stop=(cc == 1))
                nc.vector.tensor_copy(qrep[:, ncc * 512:(ncc + 1) * 512], psq[:])

            # k projection into partition strips (fp16)
            kstrip = []
            for grp in range(2):
                psk = ps_mi.tile([128, 128], F32, tag="mi", name=f"k{b}{grp}")
                for s in range(4):
                    kc = grp * 4 + s
                    for cc in range(2):
                        nc.tensor.matmul(
                            psk[32 * s:32 * (s + 1), :],
                            kw_sb[cc][:],
                            xkbn[cc][:, kc * 128:(kc + 1) * 128],
                            start=(cc == 0), stop=(cc == 1),
                            tile_position=(0, 32 * s))
                kt = pool.tile([128, 128], FP16, tag=f"ks{b}{grp}", name=f"ks{b}{grp}")
                nc.vector.tensor_copy(kt[:], psk[:])
                kstrip.append(kt)

            # v projection: all 8 kc into one [128, 256] PSUM tile, then one
            # strided copy into [128, 8*33] (+ ones columns) for AV stationary
            psv = ps_mi.tile([128, 256], F32, tag="mi", name=f"v{b}")
            for kc in range(8):
                for cc in range(2):
                    nc.tensor.matmul(
                        psv[:, kc * 32:(kc + 1) * 32],
                        xkbn[cc][:, kc * 128:(kc + 1) * 128],
                        vw_sb[cc][:],
                        start=(cc == 0), stop=(cc == 1))
            vsb = pool.tile([128, 8 * 33], FP16, tag=f"v{b}", name=f"vt{b}")
            vview = vsb[:].rearrange("p (k d) -> p k d", k=8, d=33)
            nc.vector.tensor_copy(
                vview[:, :, 0:32],
                psv[:].rearrange("p (k d) -> p k d", k=8, d=32))
            nc.vector.memset(vview[:, :, 32:33], 1.0)
            bctx[b] = (qrep, kstrip, vsb)

        # erel tiles: one [128, 4096] fp16 tile per qc (8 contiguous 128KB DMAs)
        def load_erel(qc):
            er = pool.tile([128, 8 * 512], FP16, tag=f"erel{qc % 2}", bufs=2,
                           name=f"erel{qc}")
            for kc in range(8):
                nc.sync.dma_start(er[:, kc * 512:(kc + 1) * 512], erel_d[qc, kc])
            return er

        def attn(qc, b, er, av_tile):
            qrep, kstrip, vsb = bctx[b]
            off = 64 * (b % 2)
            qsl = slice(qc * 512, (qc + 1) * 512)
            et = pool.tile([128, 4096], FP16, tag="et", bufs=2,
                           name=f"et{qc}{b}")
            for g in range(4):
                pssc = ps_sc.tile([128, 1024], F32, tag="sc", name=f"sc{qc}{b}{g}")
                for u in range(2):
                    kc = 2 * g + u
                    s = kc % 4
                    nc.tensor.matmul(
                        pssc[:, u * 512:(u + 1) * 512],
                        kstrip[kc // 4][32 * s:32 * (s + 1), :],
                        qrep[32 * s:32 * (s + 1), qsl],
                        start=True, stop=True,
                        tile_position=(32 * s, 0))
                nc.scalar.activation(et[:, g * 1024:(g + 1) * 1024], pssc[:],
                                     mybir.ActivationFunctionType.Exp)
            # one big fp16 2x-mode multiply for the whole (qc, b) tile
            nc.vector.tensor_tensor(et[:], et[:], er[:],
                                    op=mybir.AluOpType.mult)
            for kc in range(8):
                nc.tensor.matmul(
                    av_tile[off:off + 33, :],
                    vsb[:, kc * 33:kc * 33 + 33],
                    et[:, kc * 512:(kc + 1) * 512],
                    start=(kc == 0), stop=(kc == 7),
                    tile_position=(0, off), skip_group_check=True)

        def emit_out(qc, bpair, av_tile):
            ou = pool.tile([97, 512], F32, tag="outu", bufs=3, name=f"ou{qc}{bpair}")
            for b in (2 * bpair, 2 * bpair + 1):
                off = 64 * (b % 2)
                nc.vector.tensor_copy(ou[off:off + 33, :], av_tile[off:off + 33, :])
                dest = b * 2 + qc // 4
                nc.sync.dma_start(outu_d[qc % 4][dest], ou[off:off + 33, :])

        # qc iteration order: chunk k = {qc k, qc k+4} -> covers all 8 dests
        CHUNK_QCS = [(0, 4), (1, 5), (2, 6), (3, 7)]

        def attn_qc(qc, bs, er):
            for b in bs:
                if b % 2 == 0:
                    av = ps_av.tile([128, 512], F32, tag=f"av{b // 2}",
                                    name=f"av{qc}p{b // 2}")
                    attn_qc.av[b // 2] = av
                attn(qc, b, er, attn_qc.av[b // 2])
            for bpair in sorted({b // 2 for b in bs}):
                emit_out(qc, bpair, attn_qc.av[bpair])

        attn_qc.av = [None, None]

        def launch_chunk(k):
            nc.gpsimd.collective_compute(
                "AllToAll", mybir.AluOpType.bypass,
                replica_groups=[list(range(N_CORES))],
                ins=[outu_d[k].opt()], outs=[recv_d[k].opt()])

        def proj_chunk(k):
            rv = recv_d[k]
            # denominators packed [128, 32]: head s -> partitions 16s..16s+16
            den = pool.tile([128, 32], F32, tag="den", bufs=2, name=f"den{k}")
            for s in range(8):
                nc.sync.dma_start(
                    den[16 * s:16 * (s + 1), :],
                    rv[s, 32, :].rearrange("(a c) -> a c", a=16, c=32))
            recip = pool.tile([128, 32], F32, tag="recip", bufs=2, name=f"rcp{k}")
            nc.vector.reciprocal(recip[:], den[:])
            nc.sync.dma_start(recip_d[k], recip[:])

            lhs = [pool.tile([128, 512], F32, tag=f"lhs{i}", bufs=2,
                             name=f"lhs{k}{i}") for i in range(2)]
            for s in range(8):
                nc.sync.dma_start(lhs[s // 4][(s % 4) * 32:(s % 4 + 1) * 32, :],
                                  rv[s, 0:32, :])
            bcr = [pool.tile([128, 512], F32, tag=f"bcr{i}", bufs=2,
                             name=f"bcr{k}{i}") for i in range(2)]
            for s in range(8):
                nc.gpsimd.dma_start(
                    bcr[s // 4][(s % 4) * 32:(s % 4 + 1) * 32, :],
                    recip_d[k, s, :].partition_broadcast(32))
            lhsh = [pool.tile([128, 512], FP16, tag=f"lhsh{i}", bufs=2,
                              name=f"lhsh{k}{i}") for i in range(2)]
            for i in range(2):
                nc.gpsimd.tensor_tensor(lhsh[i][:], lhs[i][:], bcr[i][:],
                                        op=mybir.AluOpType.mult)
            for r in range(4):
                psp = ps_mi.tile([128, 256], F32, tag="mi", name=f"pj{k}{r}")
                for i in range(2):
                    nc.tensor.matmul(psp[:], lhsh[i][:, r * 128:(r + 1) * 128],
                                     pwt_sb[i][:],
                                     start=(i == 0), stop=(i == 1))
                ot = pool.tile([128, 256], F32, tag="ot", bufs=2,
                               name=f"ot{k}{r}")
                nc.vector.tensor_add(ot[:], psp[:], pb_sb[:])
                nc.sync.dma_start(out_d[k * 512 + r * 128:k * 512 + (r + 1) * 128, :],
                                  ot[:])

        # ---- prologue: interleave batch prep with first-chunk attention ----
        prep_batch(0)
        prep_batch(1)
        er0 = load_erel(0)
        attn_qc(0, [0, 1], er0)
        er4 = load_erel(4)
        attn_qc(4, [0, 1], er4)
        prep_batch(2)
        prep_batch(3)
        attn_qc(0, [2, 3], er0)
        attn_qc(4, [2, 3], er4)
        launch_chunk(0)

        for k in range(1, 4):
            for qc in CHUNK_QCS[k]:
                er = load_erel(qc)
                attn_qc(qc, [0, 1, 2, 3], er)
            if k >= 2:
                proj_chunk(k - 2)
            if k == 3:
                proj_chunk(2)
            launch_chunk(k)
        proj_chunk(3)

    nc.compile()
    return nc


def _host_prep(x, relative_pos, q_w, k_w, v_w, proj_w, proj_b, sr_w, sr_b,
               bn_gamma, bn_beta, bn_mean, bn_var):
    f = np.float32
    h16 = np.float16
    x = np.asarray(x, f)
    relative_pos = np.asarray(relative_pos, f)
    scale = np.float32(DH ** -0.5)

    xt = np.ascontiguousarray(x.transpose(0, 2, 1)).reshape(B, 2, 128, N)
    xt = xt.astype(h16)
    a = (np.asarray(bn_gamma, f) / np.sqrt(np.asarray(bn_var, f) + BN_EPS)).astype(f)
    b_eff = ((np.asarray(sr_b, f) - np.asarray(bn_mean, f)) * a
             + np.asarray(bn_beta, f)).astype(f)
    sr_w = np.asarray(sr_w, f)
    cw = np.zeros((4, 2, 128, 128), h16)
    for t in range(4):
        tap = sr_w[:, 0, t // 2, t % 2]
        for cc in range(2):
            cw[t, cc] = np.diag(tap[cc * 128:(cc + 1) * 128]).astype(h16)
    bna = a.reshape(2, 128, 1).astype(f)
    bnb = b_eff.reshape(2, 128, 1).astype(f)
    pwt = np.ascontiguousarray(np.asarray(proj_w, f).T).reshape(2, 128, 256)
    pwt = pwt.astype(h16)
    pbrep = np.tile(np.asarray(proj_b, f).reshape(1, 256), (128, 1))

    in_maps = []
    for h in range(N_CORES):
        qwT_rep = np.tile(
            np.ascontiguousarray((np.asarray(q_w, f)[h * 32:(h + 1) * 32, :]
                                  * scale).T), (1, 4)).reshape(2, 128, 128)
        kwT = np.ascontiguousarray(
            np.asarray(k_w, f)[h * 32:(h + 1) * 32, :].T).reshape(2, 128, 32)
        vwT = np.ascontiguousarray(
            np.asarray(v_w, f)[h * 32:(h + 1) * 32, :].T).reshape(2, 128, 32)
        # erel[qc, kc, p, c] = exp(rel[h][qc*512 + c, kc*128 + p]) in fp16
        r = relative_pos[h].reshape(8, 512, 8, 128)
        erel = np.exp(r).transpose(0, 2, 3, 1).astype(h16)
        in_maps.append({
            "xt": xt, "erel": np.ascontiguousarray(erel),
            "qw": np.ascontiguousarray(qwT_rep).astype(h16),
            "kw": kwT.astype(h16), "vw": vwT.astype(h16), "cw": cw,
            "bna": bna, "bnb": bnb,
            "pwt": np.ascontiguousarray(pwt),
            "pbrep": np.ascontiguousarray(pbrep),
        })
    return in_maps


def run_once(inputs, trace=False, trace_kwargs=None):
    if trace:
        try:
            import antenv.axon_hooks  # noqa: F401
        except ImportError:
            trace = False
    if "nc" not in _CACHE:
        _CACHE["nc"] = _build_nc()
    nc = _CACHE["nc"]
    in_maps = _host_prep(
        inputs["x"], inputs["relative_pos"], inputs["q_w"], inputs["k_w"],
        inputs["v_w"], inputs["proj_w"], inputs["proj_b"], inputs["sr_w"],
        inputs["sr_b"], inputs["bn_gamma"], inputs["bn_beta"],
        inputs["bn_mean"], inputs["bn_var"])
    res = run_bass_kernel_spmd(nc, in_maps, core_ids=list(range(N_CORES)),
                               trace=trace, **(trace_kwargs or {}))
    out = np.zeros((B, N, C), np.float32)
    for i in range(N_CORES):
        bb, nh = i // 2, i % 2
        out[bb, nh * 2048:(nh + 1) * 2048, :] = res.results[i]["out"]
    return out, res


def kernel(**inputs) -> np.ndarray:
    out, _ = run_once(inputs, trace=False)
    return out
